# revision 3
# baseline (speedup 1.0000x reference)
"""Trainium2 Bass kernel for CapsNet DigitCaps dynamic routing (3 iterations).

Reference math:
    u_hat[b,i,o,u] = sum_k W[i,o,u,k] * inp[b,i,k]
    repeat 3x: c = softmax(b, o); s = sum_i c[i,o]*u_hat; v = squash(s);
               b += mean_b sum_u u_hat * v

Zero-collective full-replication design: the whole routing computation is
tiny (5 GEMM-equivalents of ~377 MMAC), so every core redundantly computes
the full-batch routing state (s, v, b, c) and only the LAST iteration's
squash is sharded: core j contracts just its 32-row batch slice of s and
emits v[32j:32j+32].  No collectives at all -- this removes the ~60us ncfw
first-collective setup and 3 serial collective latencies that dominated the
previous (capsule-sharded, AllReduce-based) kernel.

Factored on-device form (u_hat never materialized):
    s[b,(u,o)]   = sum_{ik} inpT[ik,b]^T (wt[ik,(u,o)] * c[i,o])   # 72-chunk matmul
    delta[i,o]   = (1/B) sum_{u,k} wt[ik,uo] * (inp^T @ v)[ik,uo]
where the (u,k) reduction runs on the tensor engine: 16 accumulating
matmuls per chunk-group against a constant block-diagonal 8x8-ones/B
matrix (k8) reduce over k-groups AND broadcast back to all 8 k-rows, so b
lives in (i,k)-expanded layout [9216,10] and softmax is pure elementwise.

All (o,u)-indexed tensors are stored U-MAJOR (col = u*10+o): the broadcast
of c[.,o] over u then has a packed innermost o-dim, making the big
wc = wt*c tensor-tensor DVE-2x eligible, and the per-u slices fed to the
k8 reduction matmuls are contiguous.

Routing runs in bf16 (fp32 PSUM accumulation).  W*M2 products go through
an ACT-engine PSUM->SBUF bf16 copy for most groups (DVE-2x multiply) with
a few groups multiplied by DVE straight from PSUM (1x) to balance engines.
PE p-state warm chains (ACT<->PE ping-pong) pace dummy matmuls through the
DMA-paced iteration-0 matmul and the squash/softmax windows.
"""

import numpy as np

N_CORES = 8
B = 256
IC, OC, OU, IK = 1152, 10, 16, 8
PK = IC * IK            # 9216 (i,k) pairs, fully replicated per core
NCH = PK // 128         # 72 partition chunks
F = OC * OU             # 160 free (u,o), u-major: col = u*10 + o
BT = B // 128           # 2 batch partition tiles
BSL = B // N_CORES      # 32-row batch slice per core (final iteration)
HALF = NCH // 2         # 36 chunks per b-update half
GSZ = 3                 # chunks per M2 PSUM-bank group (3*160 fp32 = 1920B < 2KB)
NGRP = HALF // GSZ      # 12 groups per half
MAGIC = 1597463007.0    # 0x5f3759df as float (rsqrt seed)

_CACHE = {}


def _build():
    import concourse.bacc as bacc
    import concourse.mybir as mybir
    import concourse.tile as tile

    fp32 = mybir.dt.float32
    bf16 = mybir.dt.bfloat16
    i32 = mybir.dt.int32
    AF = mybir.ActivationFunctionType
    ALU = mybir.AluOpType
    AX = mybir.AxisListType

    nc = bacc.Bacc("TRN2", target_bir_lowering=False, debug=False, num_devices=N_CORES)

    wt_d = nc.dram_tensor("wt", [128, NCH, F], bf16, kind="ExternalInput")
    inpT_d = nc.dram_tensor("inpT", [128, NCH, B], bf16, kind="ExternalInput")
    inp_bk_d = nc.dram_tensor("inp_bk", [128, BT, PK], bf16, kind="ExternalInput")
    sl_d = nc.dram_tensor("inpT_sl", [128, NCH, BSL], bf16, kind="ExternalInput")
    k8_d = nc.dram_tensor("k8", [128, 128], bf16, kind="ExternalInput")
    v_d = nc.dram_tensor("v_out", [BSL, F], fp32, kind="ExternalOutput")

    with tile.TileContext(nc) as tc:
        with (
            tc.tile_pool(name="main", bufs=1) as mp,
            tc.tile_pool(name="rot", bufs=3) as rotp,
            tc.tile_pool(name="ps", bufs=3, space="PSUM") as pp,
            tc.tile_pool(name="psb", bufs=1, space="PSUM") as ppb,
        ):
            k8 = mp.tile([128, 128], bf16, tag="k8", name="k8")
            nc.gpsimd.dma_start(k8[:], k8_d[:])

            wt = mp.tile([128, NCH, F], bf16, tag="wt", name="wt")
            inpT = mp.tile([128, NCH, B], bf16, tag="inpT", name="inpT")
            inp_bk = mp.tile([128, BT, PK], bf16, tag="inp_bk", name="inp_bk")
            inpT_sl = mp.tile([128, NCH, BSL], bf16, tag="inpT_sl", name="inpT_sl")
            # chunk-sliced loads so iteration-0 matmuls can start early
            for s in range(9):
                cs = slice(8 * s, 8 * s + 8)
                nc.gpsimd.dma_start(wt[:, cs, :], wt_d[:, cs, :])
                nc.gpsimd.dma_start(inpT[:, cs, :], inpT_d[:, cs, :])
            for s in range(9):
                ks = slice(1024 * s, 1024 * s + 1024)
                nc.gpsimd.dma_start(inp_bk[:, :, ks], inp_bk_d[:, :, ks])
            nc.gpsimd.dma_start(inpT_sl[:], sl_d[:])

            scr = mp.tile([128, 8], fp32, tag="scr", name="scr")
            nc.vector.memset(scr[:], 1.0)
            # load the exp ACT table before anything needs it
            nc.scalar.activation(scr[:, 0:1], scr[:, 0:1], AF.Exp)

            # PE p-state warm chain: ACT<->PE ping-pong paces dummy matmuls
            wact = mp.tile([128, 16], bf16, tag="wact", name="wact")
            nc.vector.memset(wact[:], 1.0)
            wmp = ppb.tile([16, 16], fp32, tag="wmp", name="wmp")

            def warm_chain(n, seed_ap=None):
                if seed_ap is not None:
                    nc.scalar.mul(wact[:], wact[:], seed_ap)
                for _ in range(n):
                    nc.scalar.mul(wact[:], wact[:], 1.0)
                    nc.tensor.matmul(wmp[:], k8[:, 0:16], wact[:], start=True, stop=True)

            b_exp = mp.tile([128, NCH, OC], fp32, tag="b_exp", name="b_exp")
            e_exp = mp.tile([128, NCH, OC], fp32, tag="e_exp", name="e_exp")
            c_exp = mp.tile([128, NCH, OC], bf16, tag="c_exp", name="c_exp")
            rs = mp.tile([128, NCH], fp32, tag="rs", name="rs")
            ri = mp.tile([128, NCH], fp32, tag="ri", name="ri")
            wc = mp.tile([128, NCH, F], bf16, tag="wc", name="wc")
            v_bf = mp.tile([128, BT, F], bf16, tag="v_bf", name="v_bf")
            s_sb = mp.tile([128, BT, F], fp32, tag="s_sb", name="s_sb")

            def squash(P, NH, s_ap, v_out_ap, scale, sfx):
                """v = squash(s) over u; s_ap/v_out_ap are [P, NH, F] u-major."""
                sq = mp.tile([P, NH, F], fp32, tag=f"sq{sfx}", name=f"sq{sfx}")
                nc.scalar.activation(sq[:], s_ap, AF.Square, scale=scale)
                sqn = mp.tile([P, NH, OC], fp32, tag=f"sqn{sfx}", name=f"sqn{sfx}")
                nc.vector.reduce_sum(
                    sqn[:], sq[:].rearrange("p h (u o) -> p h o u", u=OU), axis=AX.X
                )
                xf = mp.tile([P, NH, OC], fp32, tag=f"xf{sfx}", name=f"xf{sfx}")
                nc.vector.tensor_copy(xf[:], sqn[:].bitcast(i32))
                yf = mp.tile([P, NH, OC], fp32, tag=f"yf{sfx}", name=f"yf{sfx}")
                nc.vector.tensor_scalar(
                    out=yf[:], in0=xf[:], scalar1=-0.5, scalar2=MAGIC,
                    op0=ALU.mult, op1=ALU.add,
                )
                yi = mp.tile([P, NH, OC], i32, tag=f"yi{sfx}", name=f"yi{sfx}")
                nc.vector.tensor_copy(yi[:], yf[:])
                y = yi[:].bitcast(fp32)
                t1 = mp.tile([P, NH, OC], fp32, tag=f"t1{sfx}", name=f"t1{sfx}")
                t2 = mp.tile([P, NH, OC], fp32, tag=f"t2{sfx}", name=f"t2{sfx}")
                y2 = mp.tile([P, NH, OC], fp32, tag=f"y2{sfx}", name=f"y2{sfx}")
                # Newton: y' = (1.5 - 0.5*x*y^2) * y, twice
                nc.vector.tensor_tensor(out=t1[:], in0=y, in1=y, op=ALU.mult)
                nc.vector.scalar_tensor_tensor(
                    out=t2[:], in0=t1[:], scalar=-0.5, in1=sqn[:],
                    op0=ALU.mult, op1=ALU.mult,
                )
                nc.vector.scalar_tensor_tensor(
                    out=y2[:], in0=t2[:], scalar=1.5, in1=y,
                    op0=ALU.add, op1=ALU.mult,
                )
                nc.vector.tensor_tensor(out=t1[:], in0=y2[:], in1=y2[:], op=ALU.mult)
                nc.vector.scalar_tensor_tensor(
                    out=t2[:], in0=t1[:], scalar=-0.5, in1=sqn[:],
                    op0=ALU.mult, op1=ALU.mult,
                )
                nc.vector.scalar_tensor_tensor(
                    out=y2[:], in0=t2[:], scalar=1.5, in1=y2[:],
                    op0=ALU.add, op1=ALU.mult,
                )
                # f = scale * sqn * y / (1 + sqn)
                d1 = mp.tile([P, NH, OC], fp32, tag=f"d1{sfx}", name=f"d1{sfx}")
                nc.vector.tensor_scalar_add(d1[:], sqn[:], 1.0)
                dr = mp.tile([P, NH, OC], fp32, tag=f"dr{sfx}", name=f"dr{sfx}")
                nc.vector.reciprocal(dr[:], d1[:])
                f2 = mp.tile([P, NH, OC], fp32, tag=f"f2{sfx}", name=f"f2{sfx}")
                nc.vector.tensor_tensor(out=f2[:], in0=sqn[:], in1=y2[:], op=ALU.mult)
                ff = mp.tile([P, NH, OC], fp32, tag=f"ff{sfx}", name=f"ff{sfx}")
                nc.vector.scalar_tensor_tensor(
                    out=ff[:], in0=f2[:], scalar=scale, in1=dr[:],
                    op0=ALU.mult, op1=ALU.mult,
                )
                nc.vector.tensor_tensor(
                    out=v_out_ap.rearrange("p h (u o) -> p h u o", u=OU),
                    in0=s_ap.rearrange("p h (u o) -> p h u o", u=OU),
                    in1=ff[:].unsqueeze(2).broadcast_to([P, NH, OU, OC]),
                    op=ALU.mult,
                )

            def wc_quad(c0):
                """wc[:, c0:c0+4] = wt * c  (DVE 2x: innermost packed o)."""
                nc.vector.tensor_tensor(
                    out=wc[:, c0:c0 + 4, :].rearrange("p c (u o) -> p c u o", u=OU),
                    in0=wt[:, c0:c0 + 4, :].rearrange("p c (u o) -> p c u o", u=OU),
                    in1=c_exp[:, c0:c0 + 4, :].unsqueeze(2).broadcast_to(
                        [128, 4, OU, OC]
                    ),
                    op=ALU.mult,
                )

            def m2_mm(H, g):
                """M2 chunk-group: inp^T @ v over batch, one PSUM bank."""
                m2 = pp.tile([128, GSZ, F], fp32, tag="m2", name="m2")
                k = 0
                for ci in range(GSZ):
                    ch = HALF * H + GSZ * g + ci
                    for h in range(BT):
                        nc.tensor.matmul(
                            m2[:, ci, :],
                            inp_bk[:, h, 128 * ch:128 * (ch + 1)],
                            v_bf[:, h, :],
                            start=(k == 0), stop=(k == 2 * GSZ - 1),
                            skip_group_check=True,
                        )
                        k += 1
                return m2

            def g_make(H, g, m2):
                """g = wt * M2 -> bf16 SBUF (ACT-copy + DVE-2x, or DVE direct)."""
                ch0 = HALF * H + GSZ * g
                wts = wt[:, ch0:ch0 + GSZ, :]
                gt = rotp.tile([128, GSZ, F], bf16, tag="gt", name="gt")
                if g % 4 == 3:
                    nc.vector.tensor_tensor(out=gt[:], in0=wts, in1=m2[:], op=ALU.mult)
                else:
                    m2sb = rotp.tile([128, GSZ, F], bf16, tag="m2sb", name="m2sb")
                    nc.scalar.copy(m2sb[:], m2[:])
                    nc.vector.tensor_tensor(out=gt[:], in0=wts, in1=m2sb[:], op=ALU.mult)
                return gt

            def ublk(g, gt, dacc, first, last):
                """(u,k)-reduce of g into expanded delta via 16 k8 matmuls."""
                for u in range(OU):
                    nc.tensor.matmul(
                        dacc[:, GSZ * g:GSZ * g + GSZ, :],
                        k8[:],
                        gt[:, :, 10 * u:10 * u + 10],
                        start=(first and u == 0), stop=(last and u == OU - 1),
                        skip_group_check=True,
                    )

            for it in range(3):
                # ---- s matmul: accumulate 72 chunks into one PSUM bank ----
                sps = ppb.tile([128, BT, F], fp32, tag="sps", name="sps")
                if it < 2:
                    for ch in range(NCH):
                        rhs = wt[:, ch, :] if it == 0 else wc[:, ch, :]
                        for h in range(BT):
                            nc.tensor.matmul(
                                sps[:, h, :],
                                inpT[:, ch, 128 * h:128 * (h + 1)],
                                rhs,
                                start=(ch == 0 and h == 0),
                                stop=(ch == NCH - 1 and h == BT - 1),
                                skip_group_check=True,
                            )
                        if it == 0 and ch % 8 == 7 and ch < NCH - 1:
                            warm_chain(4)  # fill DMA-paced gaps
                else:
                    # final iteration: only this core's 32-row batch slice
                    for ch in range(NCH):
                        nc.tensor.matmul(
                            sps[0:BSL, 0, :],
                            inpT_sl[:, ch, :],
                            wc[:, ch, :],
                            start=(ch == 0), stop=(ch == NCH - 1),
                            skip_group_check=True,
                        )

                # ---- squash ----
                if it < 2:
                    nc.scalar.copy(s_sb[:], sps[:])
                    warm_chain(8, seed_ap=s_sb[:, 0, 0:1])
                    squash(128, BT, s_sb[:], v_bf[:], 0.1 if it == 0 else 1.0, "a")
                else:
                    s32 = mp.tile([BSL, 1, F], fp32, tag="s32", name="s32")
                    nc.scalar.copy(s32[:, 0, :], sps[0:BSL, 0, :])
                    v32 = mp.tile([BSL, 1, F], fp32, tag="v32", name="v32")
                    squash(BSL, 1, s32[:], v32[:], 1.0, "b")
                    nc.scalar.dma_start(v_d[:], v32[:, 0, :])
                    continue

                # ---- b update: M2 = inp^T @ v, g = wt*M2, (u,k)-reduce ----
                for H in (0, 1):
                    dacc = ppb.tile([128, HALF, OC], fp32, tag=f"dacc{H}",
                                    name=f"dacc{H}")
                    m2s = {0: m2_mm(H, 0)}
                    for g in range(NGRP):
                        if g + 1 < NGRP:
                            m2s[g + 1] = m2_mm(H, g + 1)
                        gt = g_make(H, g, m2s.pop(g))
                        ublk(g, gt, dacc, first=(g == 0), last=(g == NGRP - 1))
                        # interleave previous half's wc production (next iter)
                        if H == 1 and g < 9:
                            wc_quad(4 * g)
                    Hs = slice(HALF * H, HALF * H + HALF)
                    # softmax over o in (i,k)-expanded layout
                    if it == 0:
                        nc.scalar.copy(b_exp[:, Hs, :], dacc[:])
                    else:
                        nc.vector.tensor_add(b_exp[:, Hs, :], b_exp[:, Hs, :], dacc[:])
                    nc.scalar.activation(e_exp[:, Hs, :], b_exp[:, Hs, :], AF.Exp)
                    nc.vector.reduce_sum(rs[:, Hs], e_exp[:, Hs, :], axis=AX.X)
                    nc.vector.reciprocal(ri[:, Hs], rs[:, Hs])
                    nc.vector.tensor_tensor(
                        out=c_exp[:, Hs, :],
                        in0=e_exp[:, Hs, :],
                        in1=ri[:, Hs].unsqueeze(2).broadcast_to([128, HALF, OC]),
                        op=ALU.mult,
                    )
                    if H == 1:
                        warm_chain(6, seed_ap=e_exp[:, HALF, 0:1])
                        for q in range(9):
                            wc_quad(HALF + 4 * q)

    nc.compile()
    return nc


def _prep_inputs(inp, W):
    import ml_dtypes

    bf = ml_dtypes.bfloat16
    inp = np.ascontiguousarray(inp, dtype=np.float32).reshape(B, PK)
    # W[i,o,u,k] -> wt[(i,k), (u,o)]  (u-major free dim)
    Wt = np.ascontiguousarray(
        np.asarray(W, dtype=np.float32).transpose(0, 3, 2, 1)
    ).reshape(PK, F)
    wt_d = np.ascontiguousarray(Wt.reshape(NCH, 128, F).transpose(1, 0, 2)).astype(bf)
    inpT_d = np.ascontiguousarray(
        inp.T.reshape(NCH, 128, B).transpose(1, 0, 2)
    ).astype(bf)
    inp_bk_d = np.ascontiguousarray(
        inp.reshape(BT, 128, PK).transpose(1, 0, 2)
    ).astype(bf)
    c = np.arange(128)
    k8 = ((c[:, None] // 8 == c[None, :] // 8) / float(B)).astype(bf)
    maps = []
    for j in range(N_CORES):
        maps.append({
            "wt": wt_d,
            "inpT": inpT_d,
            "inp_bk": inp_bk_d,
            "k8": k8,
            "inpT_sl": np.ascontiguousarray(inpT_d[:, :, BSL * j:BSL * (j + 1)]),
        })
    return maps


def _assemble(res):
    v = np.concatenate(
        [res.results[j]["v_out"] for j in range(N_CORES)], axis=0
    )  # [B, F] u-major
    v = v.reshape(B, OU, OC).transpose(0, 2, 1)  # -> [B, OC, OU]
    return np.ascontiguousarray(v).astype(np.float32)


def kernel(inp, W):
    from concourse.bass_utils import run_bass_kernel_spmd

    if "nc" not in _CACHE:
        _CACHE["nc"] = _build()
    nc = _CACHE["nc"]
    in_maps = _prep_inputs(inp, W)
    res = run_bass_kernel_spmd(nc, in_maps, list(range(N_CORES)))
    return _assemble(res)


# revision 11
# speedup vs baseline: 1.1031x; 1.1031x over previous
"""Trainium2 Bass kernel for CapsNet DigitCaps dynamic routing (3 iterations).

Reference math:
    u_hat[b,i,o,u] = sum_k W[i,o,u,k] * inp[b,i,k]
    repeat 3x: c = softmax(b, o); s = sum_i c[i,o]*u_hat; v = squash(s);
               b += mean_b sum_u u_hat * v

Zero-collective full-replication design: the whole routing computation is
tiny (5 GEMM-equivalents of ~377 MMAC), so every core redundantly computes
the full-batch routing state (s, v, b, c) and only the LAST iteration's
squash is sharded: core j contracts just its 32-row batch slice of s and
emits v[32j:32j+32].  No collectives at all -- this removes the ~60us ncfw
first-collective setup and 3 serial collective latencies that dominated the
previous (capsule-sharded, AllReduce-based) kernel.

Factored on-device form (u_hat never materialized):
    s[b,(u,o)]   = sum_{ik} inpT[ik,b]^T (wt[ik,(u,o)] * c[i,o])   # 72-chunk matmul
    delta[i,o]   = (1/B) sum_{u,k} wt[ik,uo] * (inp^T @ v)[ik,uo]
where the (u,k) reduction runs on the tensor engine: 16 accumulating
matmuls per chunk-group against a constant block-diagonal 8x8-ones/B
matrix (k8) reduce over k-groups AND broadcast back to all 8 k-rows, so b
lives in (i,k)-expanded layout [9216,10] and softmax is pure elementwise.

All (o,u)-indexed tensors are stored U-MAJOR (col = u*10+o): the broadcast
of c[.,o] over u then has a packed innermost o-dim, making the big
wc = wt*c tensor-tensor DVE-2x eligible, and the per-u slices fed to the
k8 reduction matmuls are contiguous.

Routing runs in bf16 (fp32 PSUM accumulation).  W*M2 products go through
an ACT-engine PSUM->SBUF bf16 copy for most groups (DVE-2x multiply) with
a few groups multiplied by DVE straight from PSUM (1x) to balance engines.
PE p-state warm chains (ACT<->PE ping-pong) pace dummy matmuls through the
DMA-paced iteration-0 matmul and the squash/softmax windows.
"""

import numpy as np

N_CORES = 8
B = 256
IC, OC, OU, IK = 1152, 10, 16, 8
PK = IC * IK            # 9216 (i,k) pairs, fully replicated per core
NCH = PK // 128         # 72 partition chunks
F = OC * OU             # 160 free (u,o), u-major: col = u*10 + o
BT = B // 128           # 2 batch partition tiles
BSL = B // N_CORES      # 32-row batch slice per core (final iteration)
HALF = NCH // 2         # 36 chunks per b-update half
GSZ = 3                 # chunks per M2 PSUM-bank group (3*160 fp32 = 1920B < 2KB)
NGRP = HALF // GSZ      # 12 groups per half
MAGIC = 1597463007.0    # 0x5f3759df as float (rsqrt seed)

_CACHE = {}


def _build():
    import concourse.bacc as bacc
    import concourse.mybir as mybir
    import concourse.tile as tile

    fp32 = mybir.dt.float32
    bf16 = mybir.dt.bfloat16
    i32 = mybir.dt.int32
    AF = mybir.ActivationFunctionType
    ALU = mybir.AluOpType
    AX = mybir.AxisListType

    nc = bacc.Bacc("TRN2", target_bir_lowering=False, debug=False, num_devices=N_CORES)

    wt_d = nc.dram_tensor("wt", [128, NCH, F], bf16, kind="ExternalInput")
    inpT_d = nc.dram_tensor("inpT", [128, NCH, B], bf16, kind="ExternalInput")
    inp_bk_d = nc.dram_tensor("inp_bk", [128, BT, PK], bf16, kind="ExternalInput")
    sl_d = nc.dram_tensor("inpT_sl", [128, NCH, BSL], bf16, kind="ExternalInput")
    k8_d = nc.dram_tensor("k8", [128, 128], bf16, kind="ExternalInput")
    v_d = nc.dram_tensor("v_out", [BSL, F], fp32, kind="ExternalOutput")

    with tile.TileContext(nc) as tc:
        with (
            tc.tile_pool(name="main", bufs=1) as mp,
            tc.tile_pool(name="rot", bufs=3) as rotp,
            tc.tile_pool(name="ps", bufs=3, space="PSUM") as pp,
            tc.tile_pool(name="psb", bufs=1, space="PSUM") as ppb,
        ):
            k8 = mp.tile([128, 128], bf16, tag="k8", name="k8")
            nc.gpsimd.dma_start(k8[:], k8_d[:])

            wt = mp.tile([128, NCH, F], bf16, tag="wt", name="wt")
            inpT = mp.tile([128, NCH, B], bf16, tag="inpT", name="inpT")
            inp_bk = mp.tile([128, BT, PK], bf16, tag="inp_bk", name="inp_bk")
            inpT_sl = mp.tile([128, NCH, BSL], bf16, tag="inpT_sl", name="inpT_sl")
            scr = mp.tile([128, 8], fp32, tag="scr", name="scr")
            nc.vector.memset(scr[:], 1.0)
            # load the exp ACT table before anything needs it
            nc.scalar.activation(scr[:, 0:1], scr[:, 0:1], AF.Exp)

            # wt/inpT stream in 12-chunk slices on two separate DGE queues so
            # iteration-0 matmuls trail the transfers; inp_bk is gated behind
            # the last inpT slice (ACT-queue dummy dep) so its transfer does
            # not steal HBM bandwidth from the s0-critical loads.
            for s in range(6):
                cs = slice(12 * s, 12 * s + 12)
                nc.gpsimd.dma_start(wt[:, cs, :], wt_d[:, cs, :])
                nc.sync.dma_start(inpT[:, cs, :], inpT_d[:, cs, :])
            nc.scalar.copy(scr[0:1, 1:2], inpT[0:1, NCH - 1, B - 1:B])
            for s in range(3):
                ks = slice(3072 * s, 3072 * s + 3072)
                nc.scalar.dma_start(inp_bk[:, :, ks], inp_bk_d[:, :, ks])
            nc.scalar.dma_start(inpT_sl[:], sl_d[:])

            b_exp = mp.tile([128, NCH, OC], fp32, tag="b_exp", name="b_exp")
            e_exp = mp.tile([128, NCH, OC], fp32, tag="e_exp", name="e_exp")
            c_exp = mp.tile([128, NCH, OC], bf16, tag="c_exp", name="c_exp")
            rs = mp.tile([128, NCH], fp32, tag="rs", name="rs")
            ri = mp.tile([128, NCH], fp32, tag="ri", name="ri")
            wc = mp.tile([128, NCH, F], bf16, tag="wc", name="wc")
            v_bf = mp.tile([128, BT, F], bf16, tag="v_bf", name="v_bf")

            def squash(P, NH, s_ap, v_out_ap, scale, sfx):
                """v = squash(s) over u; s_ap/v_out_ap are [P, NH, F] u-major.

                rsqrt via fast-inverse-sqrt seed + one Newton step, folded:
                z = sqn*y; f2 = (1.5 - 0.5*z*y)*z = sqn*y_newton; v = s*f
                with f = scale*f2/(1+sqn).
                """
                sq = mp.tile([P, NH, F], fp32, tag=f"sq{sfx}", name=f"sq{sfx}")
                nc.scalar.activation(sq[:], s_ap, AF.Square, scale=scale)
                sqn = mp.tile([P, NH, OC], fp32, tag=f"sqn{sfx}", name=f"sqn{sfx}")
                nc.vector.reduce_sum(
                    sqn[:], sq[:].rearrange("p h (u o) -> p h o u", u=OU), axis=AX.X
                )
                yi = mp.tile([P, NH, OC], i32, tag=f"yi{sfx}", name=f"yi{sfx}")
                nc.vector.tensor_scalar(
                    out=yi[:], in0=sqn[:].bitcast(i32), scalar1=-0.5, scalar2=MAGIC,
                    op0=ALU.mult, op1=ALU.add,
                )
                y = yi[:].bitcast(fp32)
                d1 = mp.tile([P, NH, OC], fp32, tag=f"d1{sfx}", name=f"d1{sfx}")
                nc.vector.tensor_scalar_add(d1[:], sqn[:], 1.0)
                dr = mp.tile([P, NH, OC], fp32, tag=f"dr{sfx}", name=f"dr{sfx}")
                nc.vector.reciprocal(dr[:], d1[:])
                t1 = mp.tile([P, NH, OC], fp32, tag=f"t1{sfx}", name=f"t1{sfx}")
                nc.vector.tensor_tensor(out=t1[:], in0=y, in1=y, op=ALU.mult)
                t2 = mp.tile([P, NH, OC], fp32, tag=f"t2{sfx}", name=f"t2{sfx}")
                nc.vector.scalar_tensor_tensor(
                    out=t2[:], in0=t1[:], scalar=-0.5, in1=sqn[:],
                    op0=ALU.mult, op1=ALU.mult,
                )
                y2 = mp.tile([P, NH, OC], fp32, tag=f"y2{sfx}", name=f"y2{sfx}")
                nc.vector.scalar_tensor_tensor(
                    out=y2[:], in0=t2[:], scalar=1.5, in1=y,
                    op0=ALU.add, op1=ALU.mult,
                )
                f2 = mp.tile([P, NH, OC], fp32, tag=f"f2{sfx}", name=f"f2{sfx}")
                nc.vector.tensor_tensor(out=f2[:], in0=sqn[:], in1=y2[:], op=ALU.mult)
                ff = mp.tile([P, NH, OC], fp32, tag=f"ff{sfx}", name=f"ff{sfx}")
                nc.vector.scalar_tensor_tensor(
                    out=ff[:], in0=f2[:], scalar=scale, in1=dr[:],
                    op0=ALU.mult, op1=ALU.mult,
                )
                nc.vector.tensor_tensor(
                    out=v_out_ap.rearrange("p h (u o) -> p h u o", u=OU),
                    in0=s_ap.rearrange("p h (u o) -> p h u o", u=OU),
                    in1=ff[:].unsqueeze(2).broadcast_to([P, NH, OU, OC]),
                    op=ALU.mult,
                )

            def wc_quad(c0):
                """wc[:, c0:c0+4] = wt * c  (DVE 2x: innermost packed o)."""
                nc.vector.tensor_tensor(
                    out=wc[:, c0:c0 + 4, :].rearrange("p c (u o) -> p c u o", u=OU),
                    in0=wt[:, c0:c0 + 4, :].rearrange("p c (u o) -> p c u o", u=OU),
                    in1=c_exp[:, c0:c0 + 4, :].unsqueeze(2).broadcast_to(
                        [128, 4, OU, OC]
                    ),
                    op=ALU.mult,
                )

            def m2_mm(H, g):
                """M2 chunk-group: inp^T @ v over batch, one PSUM bank."""
                m2 = pp.tile([128, GSZ, F], fp32, tag="m2", name="m2")
                k = 0
                for ci in range(GSZ):
                    ch = HALF * H + GSZ * g + ci
                    for h in range(BT):
                        nc.tensor.matmul(
                            m2[:, ci, :],
                            inp_bk[:, h, 128 * ch:128 * (ch + 1)],
                            v_bf[:, h, :],
                            start=(k == 0), stop=(k == 2 * GSZ - 1),
                            skip_group_check=True,
                        )
                        k += 1
                return m2

            def g_make(H, g, m2, gt):
                """g = wt * M2 -> bf16 SBUF (ACT-copy + DVE-2x, or DVE direct)."""
                ch0 = HALF * H + GSZ * g
                wts = wt[:, ch0:ch0 + GSZ, :]
                dst = gt[:, GSZ * (g % 2):GSZ * (g % 2) + GSZ, :]
                if g % 4 == 3:
                    nc.vector.tensor_tensor(out=dst, in0=wts, in1=m2[:], op=ALU.mult)
                else:
                    m2sb = rotp.tile([128, GSZ, F], bf16, tag="m2sb", name="m2sb")
                    nc.scalar.copy(m2sb[:], m2[:])
                    nc.vector.tensor_tensor(out=dst, in0=wts, in1=m2sb[:], op=ALU.mult)

            def ublk(gp, gt, dacc, first, last):
                """(u,k)-reduce of 2 groups into expanded delta, 16 k8 matmuls."""
                for u in range(OU):
                    nc.tensor.matmul(
                        dacc[:, 2 * GSZ * gp:2 * GSZ * (gp + 1), :],
                        k8[:],
                        gt[:, :, 10 * u:10 * u + 10],
                        start=(first and u == 0), stop=(last and u == OU - 1),
                        skip_group_check=True,
                    )

            for it in range(3):
                # ---- s matmul: accumulate 72 chunks into one PSUM bank ----
                sps = ppb.tile([128, BT, F], fp32, tag="sps", name="sps")
                if it < 2:
                    for ch in range(NCH):
                        rhs = wt[:, ch, :] if it == 0 else wc[:, ch, :]
                        for h in range(BT):
                            nc.tensor.matmul(
                                sps[:, h, :],
                                inpT[:, ch, 128 * h:128 * (h + 1)],
                                rhs,
                                start=(ch == 0 and h == 0),
                                stop=(ch == NCH - 1 and h == BT - 1),
                                skip_group_check=True,
                            )
                else:
                    # final iteration: only this core's 32-row batch slice
                    for ch in range(NCH):
                        nc.tensor.matmul(
                            sps[0:BSL, 0, :],
                            inpT_sl[:, ch, :],
                            wc[:, ch, :],
                            start=(ch == 0), stop=(ch == NCH - 1),
                            skip_group_check=True,
                        )

                # ---- squash (reads s straight from PSUM) ----
                if it < 2:
                    squash(128, BT, sps[:], v_bf[:], 0.1 if it == 0 else 1.0, "a")
                else:
                    v32 = mp.tile([BSL, 1, F], fp32, tag="v32", name="v32")
                    squash(BSL, 1, sps[0:BSL, 0:1, :], v32[:], 1.0, "b")
                    nc.scalar.dma_start(v_d[:], v32[:, 0, :])
                    continue

                # ---- b update: M2 = inp^T @ v, g = wt*M2, (u,k)-reduce ----
                for H in (0, 1):
                    dacc = ppb.tile([128, HALF, OC], fp32, tag=f"dacc{H}",
                                    name=f"dacc{H}")
                    m2s = {0: m2_mm(H, 0)}
                    gt = None
                    for g in range(NGRP):
                        if g + 1 < NGRP:
                            m2s[g + 1] = m2_mm(H, g + 1)
                        if g % 2 == 0:
                            gt = rotp.tile([128, 2 * GSZ, F], bf16, tag="gt",
                                           name="gt")
                        g_make(H, g, m2s.pop(g), gt)
                        if g % 2 == 1:
                            ublk(g // 2, gt, dacc,
                                 first=(g == 1), last=(g == NGRP - 1))
                        # interleave previous half's wc production (next iter)
                        if H == 1 and g < 9:
                            wc_quad(4 * g)
                    Hs = slice(HALF * H, HALF * H + HALF)
                    # softmax over o in (i,k)-expanded layout
                    if it == 0:
                        nc.scalar.copy(b_exp[:, Hs, :], dacc[:])
                    else:
                        nc.vector.tensor_add(b_exp[:, Hs, :], b_exp[:, Hs, :], dacc[:])
                    nc.scalar.activation(e_exp[:, Hs, :], b_exp[:, Hs, :], AF.Exp)
                    nc.vector.reduce_sum(rs[:, Hs], e_exp[:, Hs, :], axis=AX.X)
                    nc.vector.reciprocal(ri[:, Hs], rs[:, Hs])
                    nc.vector.tensor_tensor(
                        out=c_exp[:, Hs, :],
                        in0=e_exp[:, Hs, :],
                        in1=ri[:, Hs].unsqueeze(2).broadcast_to([128, HALF, OC]),
                        op=ALU.mult,
                    )
                    if H == 1:
                        for q in range(9):
                            wc_quad(HALF + 4 * q)

    nc.compile()
    return nc


def _prep_inputs(inp, W):
    import ml_dtypes

    bf = ml_dtypes.bfloat16
    inp = np.ascontiguousarray(inp, dtype=np.float32).reshape(B, PK)
    # W[i,o,u,k] -> wt[(i,k), (u,o)]  (u-major free dim)
    Wt = np.ascontiguousarray(
        np.asarray(W, dtype=np.float32).transpose(0, 3, 2, 1)
    ).reshape(PK, F)
    wt_d = np.ascontiguousarray(Wt.reshape(NCH, 128, F).transpose(1, 0, 2)).astype(bf)
    inpT_d = np.ascontiguousarray(
        inp.T.reshape(NCH, 128, B).transpose(1, 0, 2)
    ).astype(bf)
    inp_bk_d = np.ascontiguousarray(
        inp.reshape(BT, 128, PK).transpose(1, 0, 2)
    ).astype(bf)
    c = np.arange(128)
    k8 = ((c[:, None] // 8 == c[None, :] // 8) / float(B)).astype(bf)
    maps = []
    for j in range(N_CORES):
        maps.append({
            "wt": wt_d,
            "inpT": inpT_d,
            "inp_bk": inp_bk_d,
            "k8": k8,
            "inpT_sl": np.ascontiguousarray(inpT_d[:, :, BSL * j:BSL * (j + 1)]),
        })
    return maps


def _assemble(res):
    v = np.concatenate(
        [res.results[j]["v_out"] for j in range(N_CORES)], axis=0
    )  # [B, F] u-major
    v = v.reshape(B, OU, OC).transpose(0, 2, 1)  # -> [B, OC, OU]
    return np.ascontiguousarray(v).astype(np.float32)


def kernel(inp, W):
    from concourse.bass_utils import run_bass_kernel_spmd

    if "nc" not in _CACHE:
        _CACHE["nc"] = _build()
    nc = _CACHE["nc"]
    in_maps = _prep_inputs(inp, W)
    res = run_bass_kernel_spmd(nc, in_maps, list(range(N_CORES)))
    return _assemble(res)
